# revision 33
# baseline (speedup 1.0000x reference)
"""MinCountLoss Trainium2 Bass kernel — adaptive two-phase row sampling.

loss = sum_{b,n} relu(1 - box_sum(b, n)) over a non-negative density map
x: [32, 1, 1024, 1024] f32 with bboxes [32, 96, 4] (x1, y1, x2, y2).

The naive kernel streams all 128 MiB of f32 pixels from HBM and is pinned
at the chip HBM roofline (~35-49 us/iter for 16 MiB/core).  But the loss
only needs EXACT box sums for boxes whose sum is < 1; for everything else
"sum >= 1" suffices, and because the density map is non-negative, a
partial sum >= 1 PROVES the full sum is >= 1.  So:

  Phase 1 (device): read every 16th row of each image (1 MiB/core instead
    of 16 MiB/core).  For each box, compute the partial sum s1 over its
    sampled rows with the masked-row-matmul + column-mask-reduce pipeline:
      - sampled rows of 2 images pack one [128, 1024] SBUF tile (partition
        p holds image pair-half p//64, row 16*(p%64)); SWDGE casts
        f32->bf16 inside the DMA so no engine spends time casting.
      - row masks ym[p, n] and column masks cm[n, w] are precomputed once
        per launch (they depend only on bboxes).
      - A2[n, w] = sum_p ym[p, n] x[p, w] on TensorE (PSUM f32), then
        box partials via cm-masked accumulating reduce on DVE (3 images
        routed PSUM->bf16 on ScalarE first, 1 reduced straight from PSUM,
        balancing ACT/DVE at ~3 us each).
  Host glue (index plumbing only): valid boxes with s1 < 1.5 are flagged
    (margin absorbs bf16 slop; unflagged valid boxes are provably >= 1 so
    contribute exactly 0); invalid boxes contribute exactly 1.  Builds the
    phase-2 row-gather lists.
  Phase 2 (device): indirect-DMA gather of the flagged boxes' rows (f32,
    exact), mask each row to its box's x-range, reduce to exact box sums.
    Caps (128 rows / 16 boxes per core per batch) loop if ever exceeded —
    the graded workload needs 1 batch (11 flagged boxes, <=25 rows/core).

Correct for ANY non-negative density map; only the phase-2 volume is
data-dependent.  Total HBM traffic ~9 MiB vs 128 MiB for the full read.

Measured (pipelined-slope, all 8 cores sustained): phase1 ~4.3 us/iter
(DMA floor 3.3 us; DVE pair-sums columns 2:1 before the matmul so the
whole epilogue runs at half width — ACT ~2.9 us of PSUM->bf16 copies,
DVE ~3.1 us of pre-sums + masked reduces, all under the DMA) + phase2
~2.4-2.8 us (32-row indirect gather is descriptor-latency-bound), total
~7 us vs 49 us sustained (35 us paired-K) for the full-read kernel.
The pair-interior column masks keep the proof sound (s1p <= s1), at the
cost of one extra flagged box on the graded input.
"""

import numpy as np

B = 32
H = 1024
W = 1024
N = 96
N_CORES = 8
B_PER_CORE = B // N_CORES
P = 128
STRIDE = 16
SROWS = H // STRIDE          # 64 sampled rows per image
IMPP = P // SROWS            # 2 images per 128-partition tile
FLAG_TH = 1.5                # flag s1 below this (>=1 + bf16 margin)
MAXR = 32                    # gathered rows per core per phase-2 batch
MAXB = 16                    # box slots per core per phase-2 batch

_CACHE = {}


def _build_phase1(repeat=1, stage="full", xmode="pair", xbufs=3, ebufs=3,
                  pbufs=4):
    key = ("p1", repeat, stage, xmode, xbufs, ebufs, pbufs)
    if key in _CACHE:
        return _CACHE[key]

    from contextlib import ExitStack

    import concourse.bass as bass
    import concourse.tile as tile
    from concourse import bacc, mybir

    f32 = mybir.dt.float32
    bf16 = mybir.dt.bfloat16
    i32 = mybir.dt.int32
    Alu = mybir.AluOpType

    nc = bacc.Bacc(None, target_bir_lowering=False, debug=False)

    x_ext = nc.dram_tensor("x", [B_PER_CORE, H, W], f32, kind="ExternalInput").ap()
    bb_ext = nc.dram_tensor("bb", [B_PER_CORE, N, 4], i32, kind="ExternalInput").ap()
    s1_ext = nc.dram_tensor("s1", [N, B_PER_CORE], f32, kind="ExternalOutput").ap()

    with tile.TileContext(nc) as tc, ExitStack() as ctx:
        const = ctx.enter_context(tc.tile_pool(name="const", bufs=1))
        xpool = ctx.enter_context(tc.tile_pool(name="x", bufs=xbufs))
        epool = ctx.enter_context(tc.tile_pool(name="epi", bufs=ebufs))
        psum = ctx.enter_context(tc.tile_pool(name="psum", bufs=pbufs,
                                              space="PSUM"))

        # --- constants (once per launch, hidden behind the first loads) ---
        # iota2[p, w2] = 2*w2 + 0.5 (column-PAIR coordinates)
        iotaw_i = const.tile([P, W // 2], i32)
        nc.gpsimd.iota(iotaw_i[:], [[2, W // 2]], channel_multiplier=0)
        iota2 = const.tile([P, W // 2], f32)
        nc.vector.tensor_scalar(
            out=iota2[:], in0=iotaw_i[:], scalar1=0.5, scalar2=None, op0=Alu.add
        )
        # iotar[p, 1] = 16*p  (the in-pair row index of partition p)
        iotar_i = const.tile([P, 1], i32)
        nc.gpsimd.iota(iotar_i[:], [[1, 1]], channel_multiplier=STRIDE)
        iotar = const.tile([P, 1], f32)
        nc.vector.tensor_copy(out=iotar[:], in_=iotar_i[:])

        # bboxes broadcast to every partition (one contiguous DMA + cast)
        nbb = B_PER_CORE * N * 4
        bb_bc_i = const.tile([P, nbb], i32)
        bb_flat = bass.AP(tensor=bb_ext.tensor, offset=bb_ext.offset,
                          ap=[[0, P], [1, nbb]])
        nc.gpsimd.dma_start(out=bb_bc_i[:], in_=bb_flat)
        bb_bc = const.tile([P, nbb], f32)
        nc.vector.tensor_copy(out=bb_bc[:], in_=bb_bc_i[:])
        bbv = bb_bc[:].rearrange("p (b n c) -> p b n c", b=B_PER_CORE, c=4)

        # bboxes with n on partitions (for the column masks)
        bbA_i = const.tile([N, B_PER_CORE, 4], i32)
        bbA_src = bass.AP(tensor=bb_ext.tensor, offset=bb_ext.offset,
                          ap=[[4, N], [N * 4, B_PER_CORE], [1, 4]])
        nc.sync.dma_start(out=bbA_i[:], in_=bbA_src)
        bbA = const.tile([N, B_PER_CORE, 4], f32)
        nc.vector.tensor_copy(out=bbA[:], in_=bbA_i[:])

        # Row masks ym_all[p, b, n] = (y1 + 1024h <= 16p < y2 + 1024h),
        # h = b % IMPP the image's half of its pair tile.
        ym_all = const.tile([P, B_PER_CORE, N], bf16)
        for b in range(B_PER_CORE):
            h = b % IMPP
            y1v = bbv[:, b, :, 1]
            y2v = bbv[:, b, :, 3]
            if h:
                y1s = epool.tile([P, N], f32, tag="y1s")
                nc.vector.tensor_scalar(
                    out=y1s[:], in0=y1v, scalar1=float(1024 * h), scalar2=None,
                    op0=Alu.add)
                y2s = epool.tile([P, N], f32, tag="y2s")
                nc.vector.tensor_scalar(
                    out=y2s[:], in0=y2v, scalar1=float(1024 * h), scalar2=None,
                    op0=Alu.add)
                y1v, y2v = y1s[:], y2s[:]
            c2 = epool.tile([P, N], f32, tag="c2")
            nc.vector.tensor_scalar(
                out=c2[:], in0=y2v, scalar1=iotar[:], scalar2=None, op0=Alu.is_gt)
            nc.vector.scalar_tensor_tensor(
                out=ym_all[:, b, :], in0=y1v, scalar=iotar[:], in1=c2[:],
                op0=Alu.is_le, op1=Alu.mult)

        # Pair-interior column masks: cm_all[n, b, w2] = 1 iff BOTH columns
        # 2*w2, 2*w2+1 lie in [x1, x2), i.e. |2w2+0.5 - (x1+x2-1)/2| <
        # (x2-x1)/2.  Interior-only => the partial sum s1p <= s1, so
        # "s1p >= TH proves contribution 0" stays sound; the two lost edge
        # columns just flag a box or two more (sim: 12 vs 11 flagged).
        cm_all = const.tile([N, B_PER_CORE, W // 2], bf16)
        for b in range(B_PER_CORE):
            x2m1 = epool.tile([N, 1], f32, tag="x2m1")
            nc.vector.tensor_scalar(
                out=x2m1[:], in0=bbA[:, b, 2:3], scalar1=-1.0, scalar2=None,
                op0=Alu.add)
            mxn = epool.tile([N, 1], f32, tag="mxn")
            nc.vector.tensor_scalar(
                out=mxn[:], in0=bbA[:, b, 0:1], scalar1=x2m1[:],
                scalar2=-0.5, op0=Alu.add, op1=Alu.mult)
            rx = epool.tile([N, 1], f32, tag="rx")
            nc.vector.tensor_scalar(
                out=rx[:], in0=bbA[:, b, 2:3], scalar1=bbA[:, b, 0:1],
                scalar2=0.5, op0=Alu.subtract, op1=Alu.mult)
            tcm = epool.tile([N, W // 2], f32, tag="tcm")
            nc.scalar.activation(
                out=tcm[:], in_=iota2[0:N, :],
                func=mybir.ActivationFunctionType.Abs, bias=mxn[:], scale=1.0)
            nc.vector.tensor_scalar(
                out=cm_all[:, b, :], in0=tcm[:], scalar1=rx[:], scalar2=None,
                op0=Alu.is_lt)

        s1t = const.tile([N, B_PER_CORE], f32)

        npair = B_PER_CORE // IMPP
        for it in range(repeat):
            # sampled rows: partition p, pair g holds global row 16p of
            # pair block g; cast-DMA granularity set by xmode.
            xga = xpool.tile([P, npair, W], bf16, tag="xg")
            if xmode == "one":
                src = bass.AP(
                    tensor=x_ext.tensor, offset=x_ext.offset,
                    ap=[[STRIDE * W, P], [IMPP * H * W, npair], [1, W]])
                nc.gpsimd.dma_start(out=xga[:], in_=src)
            elif xmode == "pair":
                for g in range(npair):
                    src = bass.AP(
                        tensor=x_ext.tensor,
                        offset=x_ext.offset + g * IMPP * H * W,
                        ap=[[STRIDE * W, P], [1, W]])
                    nc.gpsimd.dma_start(out=xga[:, g, :], in_=src)
            elif xmode == "pairhalf":
                for g in range(npair):
                    for hh in range(2):
                        src = bass.AP(
                            tensor=x_ext.tensor,
                            offset=x_ext.offset + g * IMPP * H * W + hh * 512,
                            ap=[[STRIDE * W, P], [1, 512]])
                        nc.gpsimd.dma_start(
                            out=xga[:, g, hh * 512:(hh + 1) * 512], in_=src)
            if stage == "dma":
                nc.vector.tensor_copy(out=s1t[:, 0:1], in_=xga[0:N, 0, 0:1])
                continue

            # Pair-sum columns on DVE: xp[p, g, w2] = x[p,2w2] + x[p,2w2+1]
            # halves A2 and every epilogue op downstream.
            xp = xpool.tile([P, npair, W // 2], bf16, tag="xp")
            for g in range(npair):
                xv2 = xga[:, g, :].rearrange("p (w two) -> p w two", two=2)
                nc.vector.tensor_tensor(
                    out=xp[:, g, :], in0=xv2[:, :, 0:1], in1=xv2[:, :, 1:2],
                    op=Alu.add)
            # all 4 matmuls first (one PSUM bank each), then epilogues:
            # ACT never waits on a matmul mid-stream
            A2s = {}
            for b in range(B_PER_CORE):
                A2 = psum.tile([N, W // 2], mybir.dt.float32, tag="A2")
                A2s[b] = A2
                nc.tensor.matmul(
                    A2[:], lhsT=ym_all[:, b, :], rhs=xp[:, b // IMPP, :],
                    start=True, stop=True)
            if stage == "mm":
                for b in range(B_PER_CORE):
                    nc.vector.tensor_copy(out=s1t[:, b:b + 1],
                                          in_=A2s[b][:, 0:1])
                continue
            for b in range(B_PER_CORE):
                # PSUM -> bf16 on ScalarE, 2x-mode masked reduce on DVE
                a2b = epool.tile([N, W // 2], bf16, tag="a2b")
                nc.scalar.activation(
                    out=a2b[:], in_=A2s[b][:],
                    func=mybir.ActivationFunctionType.Copy)
                scr = epool.tile([N, W // 2], bf16, tag="scr")
                nc.vector.scalar_tensor_tensor(
                    out=scr[:], in0=a2b[:], scalar=1.0,
                    in1=cm_all[:, b, :], op0=Alu.mult, op1=Alu.mult,
                    accum_out=s1t[:, b:b + 1])

        nc.sync.dma_start(out=s1_ext[:], in_=s1t[:])

    nc.compile()
    _CACHE[key] = nc
    return nc


def _build_phase2(repeat=1, stage="full"):
    key = ("p2", repeat, stage)
    if key in _CACHE:
        return _CACHE[key]

    from contextlib import ExitStack

    import concourse.bass as bass
    import concourse.tile as tile
    from concourse import bacc, mybir

    f32 = mybir.dt.float32
    i32 = mybir.dt.int32
    Alu = mybir.AluOpType

    nc = bacc.Bacc(None, target_bir_lowering=False, debug=False)

    x_ext = nc.dram_tensor("x", [B_PER_CORE, H, W], f32, kind="ExternalInput").ap()
    ridx_ext = nc.dram_tensor("ridx", [MAXR, 1], i32, kind="ExternalInput").ap()
    cpar_ext = nc.dram_tensor("cpar", [MAXR, 2], f32, kind="ExternalInput").ap()
    ymap_ext = nc.dram_tensor("ymap", [MAXR, MAXB], f32, kind="ExternalInput").ap()
    s2_ext = nc.dram_tensor("s2", [MAXB, 1], f32, kind="ExternalOutput").ap()

    with tile.TileContext(nc) as tc, ExitStack() as ctx:
        const = ctx.enter_context(tc.tile_pool(name="const", bufs=1))
        xpool = ctx.enter_context(tc.tile_pool(name="x", bufs=3))
        epool = ctx.enter_context(tc.tile_pool(name="epi", bufs=2))
        psum = ctx.enter_context(tc.tile_pool(name="psum", bufs=2, space="PSUM"))

        iotaw_i = const.tile([P, W], i32)
        nc.gpsimd.iota(iotaw_i[:], [[1, W]], channel_multiplier=0)
        iotaw = const.tile([P, W], f32)
        nc.vector.tensor_scalar(
            out=iotaw[:], in0=iotaw_i[:], scalar1=0.5, scalar2=None, op0=Alu.add)

        ridx = const.tile([MAXR, 1], i32)
        nc.sync.dma_start(out=ridx[:], in_=ridx_ext)
        cpar = const.tile([MAXR, 2], f32)
        nc.sync.dma_start(out=cpar[:], in_=cpar_ext)
        ymap = const.tile([MAXR, MAXB], f32)
        nc.sync.dma_start(out=ymap[:], in_=ymap_ext)

        s2t = const.tile([MAXB, 1], f32)
        x_rows = bass.AP(tensor=x_ext.tensor, offset=0,
                         ap=[[W, B_PER_CORE * H], [1, W]])

        # per-row column mask is constant per launch: build it once
        # (hidden under the first gather), not inside the repeat loop
        tcm = const.tile([MAXR, W], f32)
        nc.scalar.activation(
            out=tcm[:], in_=iotaw[0:MAXR, :],
            func=mybir.ActivationFunctionType.Abs, bias=cpar[:, 0:1],
            scale=1.0)
        cm2 = const.tile([MAXR, W], f32)
        nc.vector.tensor_scalar(
            out=cm2[:], in0=tcm[:], scalar1=cpar[:, 1:2], scalar2=None,
            op0=Alu.is_lt)

        for it in range(repeat):
            xg = xpool.tile([MAXR, W], f32, tag="xg")
            nc.gpsimd.indirect_dma_start(
                out=xg[:], out_offset=None,
                in_=x_rows,
                in_offset=bass.IndirectOffsetOnAxis(ap=ridx[:, 0:1], axis=0))
            if stage == "gather":
                nc.vector.tensor_copy(out=s2t[:, 0:1], in_=xg[0:MAXB, 0:1])
                continue
            scr = epool.tile([MAXR, W], f32, tag="scr")
            rowsum = epool.tile([MAXR, 1], f32, tag="rowsum")
            nc.vector.scalar_tensor_tensor(
                out=scr[:], in0=cm2[:], scalar=1.0, in1=xg[:],
                op0=Alu.mult, op1=Alu.mult, accum_out=rowsum[:])
            s2p = psum.tile([MAXB, 1], mybir.dt.float32, tag="s2p")
            nc.tensor.matmul(s2p[:], lhsT=ymap[:], rhs=rowsum[:],
                             start=True, stop=True)
            nc.vector.tensor_copy(out=s2t[:], in_=s2p[:])

        nc.sync.dma_start(out=s2_ext[:], in_=s2t[:])

    nc.compile()
    _CACHE[key] = nc
    return nc


def _phase2_batches(bbc, valid, s1_all):
    """Pack flagged boxes into per-core phase-2 batches (index plumbing).

    Returns (n_batches, batches) where batches[k][core] =
    (ridx [MAXR,1] i32, cpar [MAXR,2] f32, ymap [MAXR,MAXB] f32,
     slots: list of (b_global, n) per slot).
    """
    x1, y1 = bbc[..., 0], bbc[..., 1]
    x2, y2 = bbc[..., 2], bbc[..., 3]
    flag = valid & (s1_all < FLAG_TH)

    per_core = []
    for core in range(N_CORES):
        items = []  # (b_global, n, rows, mxn, rx)
        for bl in range(B_PER_CORE):
            bg = core * B_PER_CORE + bl
            for n in np.nonzero(flag[bg])[0]:
                rows = np.arange(y1[bg, n], y2[bg, n], dtype=np.int64)
                items.append((bg, n, bl * H + rows,
                              -(x1[bg, n] + x2[bg, n]) / 2.0,
                              (x2[bg, n] - x1[bg, n]) / 2.0))
        per_core.append(items)

    batches = []
    cursors = [0] * N_CORES            # (item_idx, row_offset) flattened
    row_off = [0] * N_CORES
    while True:
        any_left = any(cursors[c] < len(per_core[c]) for c in range(N_CORES))
        if not any_left and batches:
            break
        batch = []
        for core in range(N_CORES):
            ridx = np.zeros((MAXR, 1), np.int32)
            cpar = np.zeros((MAXR, 2), np.float32)
            cpar[:, 1] = -1.0          # pad rows mask to all-zero
            ymap = np.zeros((MAXR, MAXB), np.float32)
            slots = []
            r = 0
            while cursors[core] < len(per_core[core]) and len(slots) < MAXB:
                bg, n, rows, mxn, rx = per_core[core][cursors[core]]
                take = rows[row_off[core]:]
                take = take[:MAXR - r]
                if len(take) == 0:
                    break
                j = len(slots)
                slots.append((bg, n))
                ridx[r:r + len(take), 0] = take
                cpar[r:r + len(take), 0] = mxn
                cpar[r:r + len(take), 1] = rx
                ymap[r:r + len(take), j] = 1.0
                r += len(take)
                row_off[core] += len(take)
                if row_off[core] >= len(rows):
                    cursors[core] += 1
                    row_off[core] = 0
                if r >= MAXR:
                    break
            batch.append((ridx, cpar, ymap, slots))
        batches.append(batch)
        if not any_left:
            break
    return len(batches), batches


def run(output, bboxes, trace=False):
    """Run the two-phase SPMD kernel; returns (loss_scalar, results)."""
    from concourse.bass_utils import run_bass_kernel_spmd

    x_all = np.ascontiguousarray(
        output.reshape(B, H, W).astype(np.float32, copy=False))
    bb_all = np.ascontiguousarray(bboxes.astype(np.int32, copy=False))

    # --- phase 1: partial sums over sampled rows ---
    nc1 = _build_phase1()
    in_maps = []
    for i in range(N_CORES):
        sl = slice(i * B_PER_CORE, (i + 1) * B_PER_CORE)
        in_maps.append({"x": np.ascontiguousarray(x_all[sl]),
                        "bb": np.ascontiguousarray(bb_all[sl])})
    res1 = run_bass_kernel_spmd(nc1, in_maps, core_ids=list(range(N_CORES)),
                                trace=trace)
    # s1 per core: [N, B_PER_CORE] -> global [B, N]
    s1_all = np.concatenate(
        [res1.results[i]["s1"].T for i in range(N_CORES)], axis=0)

    # --- host glue: classify boxes, build phase-2 gather lists ---
    bbc = np.clip(bb_all.astype(np.int64), 0, W)
    valid = (bbc[..., 2] > bbc[..., 0]) & (bbc[..., 3] > bbc[..., 1])
    contrib = np.where(valid, 0.0, 1.0)

    n_batches, batches = _phase2_batches(bbc, valid, s1_all)

    # --- phase 2: exact sums for flagged boxes ---
    nc2 = _build_phase2()
    s2_acc = {}
    for k in range(n_batches):
        in_maps2 = []
        for core in range(N_CORES):
            ridx, cpar, ymap, _ = batches[k][core]
            in_maps2.append({
                "x": in_maps[core]["x"],
                "ridx": ridx, "cpar": cpar, "ymap": ymap})
        res2 = run_bass_kernel_spmd(nc2, in_maps2,
                                    core_ids=list(range(N_CORES)), trace=False)
        for core in range(N_CORES):
            s2 = res2.results[core]["s2"][:, 0]
            for j, (bg, n) in enumerate(batches[k][core][3]):
                s2_acc[(bg, n)] = s2_acc.get((bg, n), 0.0) + float(s2[j])

    for (bg, n), s in s2_acc.items():
        contrib[bg, n] = max(1.0 - s, 0.0)

    total = np.float32(contrib.sum(dtype=np.float64))
    return np.array(total, dtype=np.float32), (res1, n_batches)


def kernel(output, bboxes):
    loss, _ = run(output, bboxes, trace=False)
    return loss


# revision 34
# speedup vs baseline: 1.0148x; 1.0148x over previous
"""MinCountLoss Trainium2 Bass kernel — adaptive two-phase row sampling.

loss = sum_{b,n} relu(1 - box_sum(b, n)) over a non-negative density map
x: [32, 1, 1024, 1024] f32 with bboxes [32, 96, 4] (x1, y1, x2, y2).

The naive kernel streams all 128 MiB of f32 pixels from HBM and is pinned
at the chip HBM roofline (~35-49 us/iter for 16 MiB/core).  But the loss
only needs EXACT box sums for boxes whose sum is < 1; for everything else
"sum >= 1" suffices, and because the density map is non-negative, a
partial sum >= 1 PROVES the full sum is >= 1.  So:

  Phase 1 (device): read every 16th row of each image (1 MiB/core instead
    of 16 MiB/core).  For each box, compute the partial sum s1 over its
    sampled rows with the masked-row-matmul + column-mask-reduce pipeline:
      - sampled rows of 2 images pack one [128, 1024] SBUF tile (partition
        p holds image pair-half p//64, row 16*(p%64)); SWDGE casts
        f32->bf16 inside the DMA so no engine spends time casting.
      - row masks ym[p, n] and column masks cm[n, w] are precomputed once
        per launch (they depend only on bboxes).
      - A2[n, w] = sum_p ym[p, n] x[p, w] on TensorE (PSUM f32), then
        box partials via cm-masked accumulating reduce on DVE (3 images
        routed PSUM->bf16 on ScalarE first, 1 reduced straight from PSUM,
        balancing ACT/DVE at ~3 us each).
  Host glue (index plumbing only): valid boxes with s1 < 1.5 are flagged
    (margin absorbs bf16 slop; unflagged valid boxes are provably >= 1 so
    contribute exactly 0); invalid boxes contribute exactly 1.  Builds the
    phase-2 row-gather lists.
  Phase 2 (device): indirect-DMA gather of the flagged boxes' rows (f32,
    exact), mask each row to its box's x-range, reduce to exact box sums.
    Caps (128 rows / 16 boxes per core per batch) loop if ever exceeded —
    the graded workload needs 1 batch (11 flagged boxes, <=25 rows/core).

Correct for ANY non-negative density map; only the phase-2 volume is
data-dependent.  Total HBM traffic ~9 MiB vs 128 MiB for the full read.

Measured (pipelined-slope, all 8 cores sustained): phase1 ~4.3 us/iter
(DMA floor 3.3 us; DVE pair-sums columns 2:1 before the matmul so the
whole epilogue runs at half width — ACT ~2.9 us of PSUM->bf16 copies,
DVE ~3.1 us of pre-sums + masked reduces, all under the DMA) + phase2
~2.4-2.8 us (32-row indirect gather is descriptor-latency-bound), total
~7 us vs 49 us sustained (35 us paired-K) for the full-read kernel.
The pair-interior column masks keep the proof sound (s1p <= s1), at the
cost of one extra flagged box on the graded input.
"""

import numpy as np

B = 32
H = 1024
W = 1024
N = 96
N_CORES = 8
B_PER_CORE = B // N_CORES
P = 128
STRIDE = 16
SROWS = H // STRIDE          # 64 sampled rows per image
IMPP = P // SROWS            # 2 images per 128-partition tile
FLAG_TH = 1.5                # flag s1 below this (>=1 + bf16 margin)
MAXR = 32                    # gathered rows per core per phase-2 batch
MAXB = 16                    # box slots per core per phase-2 batch

_CACHE = {}


def _build_phase1(repeat=1, stage="full", xmode="pair", xbufs=3, ebufs=3,
                  pbufs=4):
    key = ("p1", repeat, stage, xmode, xbufs, ebufs, pbufs)
    if key in _CACHE:
        return _CACHE[key]

    from contextlib import ExitStack

    import concourse.bass as bass
    import concourse.tile as tile
    from concourse import bacc, mybir

    f32 = mybir.dt.float32
    bf16 = mybir.dt.bfloat16
    i32 = mybir.dt.int32
    Alu = mybir.AluOpType

    nc = bacc.Bacc(None, target_bir_lowering=False, debug=False)

    x_ext = nc.dram_tensor("x", [B_PER_CORE, H, W], f32, kind="ExternalInput").ap()
    bb_ext = nc.dram_tensor("bb", [B_PER_CORE, N, 4], i32, kind="ExternalInput").ap()
    s1_ext = nc.dram_tensor("s1", [N, B_PER_CORE], f32, kind="ExternalOutput").ap()

    with tile.TileContext(nc) as tc, ExitStack() as ctx:
        const = ctx.enter_context(tc.tile_pool(name="const", bufs=1))
        xpool = ctx.enter_context(tc.tile_pool(name="x", bufs=xbufs))
        epool = ctx.enter_context(tc.tile_pool(name="epi", bufs=ebufs))
        psum = ctx.enter_context(tc.tile_pool(name="psum", bufs=pbufs,
                                              space="PSUM"))

        # --- constants (once per launch, hidden behind the first loads) ---
        # iota2[p, w2] = 2*w2 + 0.5 (column-PAIR coordinates)
        iotaw_i = const.tile([P, W // 2], i32)
        nc.gpsimd.iota(iotaw_i[:], [[2, W // 2]], channel_multiplier=0)
        iota2 = const.tile([P, W // 2], f32)
        nc.vector.tensor_scalar(
            out=iota2[:], in0=iotaw_i[:], scalar1=0.5, scalar2=None, op0=Alu.add
        )
        # iotar[p, 1] = 16*p  (the in-pair row index of partition p)
        iotar_i = const.tile([P, 1], i32)
        nc.gpsimd.iota(iotar_i[:], [[1, 1]], channel_multiplier=STRIDE)
        iotar = const.tile([P, 1], f32)
        nc.vector.tensor_copy(out=iotar[:], in_=iotar_i[:])

        # bboxes broadcast to every partition (one contiguous DMA + cast)
        nbb = B_PER_CORE * N * 4
        bb_bc_i = const.tile([P, nbb], i32)
        bb_flat = bass.AP(tensor=bb_ext.tensor, offset=bb_ext.offset,
                          ap=[[0, P], [1, nbb]])
        nc.gpsimd.dma_start(out=bb_bc_i[:], in_=bb_flat)
        bb_bc = const.tile([P, nbb], f32)
        nc.vector.tensor_copy(out=bb_bc[:], in_=bb_bc_i[:])
        bbv = bb_bc[:].rearrange("p (b n c) -> p b n c", b=B_PER_CORE, c=4)

        # bboxes with n on partitions (for the column masks)
        bbA_i = const.tile([N, B_PER_CORE, 4], i32)
        bbA_src = bass.AP(tensor=bb_ext.tensor, offset=bb_ext.offset,
                          ap=[[4, N], [N * 4, B_PER_CORE], [1, 4]])
        nc.sync.dma_start(out=bbA_i[:], in_=bbA_src)
        bbA = const.tile([N, B_PER_CORE, 4], f32)
        nc.vector.tensor_copy(out=bbA[:], in_=bbA_i[:])

        # Row masks ym_all[p, b, n] = (y1 + 1024h <= 16p < y2 + 1024h),
        # h = b % IMPP the image's half of its pair tile.
        ym_all = const.tile([P, B_PER_CORE, N], bf16)
        for b in range(B_PER_CORE):
            h = b % IMPP
            y1v = bbv[:, b, :, 1]
            y2v = bbv[:, b, :, 3]
            if h:
                y1s = epool.tile([P, N], f32, tag="y1s")
                nc.vector.tensor_scalar(
                    out=y1s[:], in0=y1v, scalar1=float(1024 * h), scalar2=None,
                    op0=Alu.add)
                y2s = epool.tile([P, N], f32, tag="y2s")
                nc.vector.tensor_scalar(
                    out=y2s[:], in0=y2v, scalar1=float(1024 * h), scalar2=None,
                    op0=Alu.add)
                y1v, y2v = y1s[:], y2s[:]
            c2 = epool.tile([P, N], f32, tag="c2")
            nc.vector.tensor_scalar(
                out=c2[:], in0=y2v, scalar1=iotar[:], scalar2=None, op0=Alu.is_gt)
            nc.vector.scalar_tensor_tensor(
                out=ym_all[:, b, :], in0=y1v, scalar=iotar[:], in1=c2[:],
                op0=Alu.is_le, op1=Alu.mult)

        # Pair-interior column masks: cm_all[n, b, w2] = 1 iff BOTH columns
        # 2*w2, 2*w2+1 lie in [x1, x2), i.e. |2w2+0.5 - (x1+x2-1)/2| <
        # (x2-x1)/2.  Interior-only => the partial sum s1p <= s1, so
        # "s1p >= TH proves contribution 0" stays sound; the two lost edge
        # columns just flag a box or two more (sim: 12 vs 11 flagged).
        cm_all = const.tile([N, B_PER_CORE, W // 2], bf16)
        for b in range(B_PER_CORE):
            x2m1 = epool.tile([N, 1], f32, tag="x2m1")
            nc.vector.tensor_scalar(
                out=x2m1[:], in0=bbA[:, b, 2:3], scalar1=-1.0, scalar2=None,
                op0=Alu.add)
            mxn = epool.tile([N, 1], f32, tag="mxn")
            nc.vector.tensor_scalar(
                out=mxn[:], in0=bbA[:, b, 0:1], scalar1=x2m1[:],
                scalar2=-0.5, op0=Alu.add, op1=Alu.mult)
            rx = epool.tile([N, 1], f32, tag="rx")
            nc.vector.tensor_scalar(
                out=rx[:], in0=bbA[:, b, 2:3], scalar1=bbA[:, b, 0:1],
                scalar2=0.5, op0=Alu.subtract, op1=Alu.mult)
            tcm = epool.tile([N, W // 2], f32, tag="tcm")
            nc.scalar.activation(
                out=tcm[:], in_=iota2[0:N, :],
                func=mybir.ActivationFunctionType.Abs, bias=mxn[:], scale=1.0)
            nc.vector.tensor_scalar(
                out=cm_all[:, b, :], in0=tcm[:], scalar1=rx[:], scalar2=None,
                op0=Alu.is_lt)

        s1t = const.tile([N, B_PER_CORE], f32)

        npair = B_PER_CORE // IMPP
        for it in range(repeat):
            # sampled rows: partition p, pair g holds global row 16p of
            # pair block g; load path set by xmode.
            if xmode == "hw":
                # f32 on the two HWDGE rings (lower fixed latency than
                # SWDGE, no Q7); the pair-sum below casts to bf16 free.
                xga = xpool.tile([P, npair, W], f32, tag="xg")
                for g in range(npair):
                    src = bass.AP(
                        tensor=x_ext.tensor,
                        offset=x_ext.offset + g * IMPP * H * W,
                        ap=[[STRIDE * W, P], [1, W]])
                    eng = nc.sync if g == 0 else nc.scalar
                    eng.dma_start(out=xga[:, g, :], in_=src)
            else:
                xga = xpool.tile([P, npair, W], bf16, tag="xg")
            if xmode == "one":
                src = bass.AP(
                    tensor=x_ext.tensor, offset=x_ext.offset,
                    ap=[[STRIDE * W, P], [IMPP * H * W, npair], [1, W]])
                nc.gpsimd.dma_start(out=xga[:], in_=src)
            elif xmode == "pair":
                for g in range(npair):
                    src = bass.AP(
                        tensor=x_ext.tensor,
                        offset=x_ext.offset + g * IMPP * H * W,
                        ap=[[STRIDE * W, P], [1, W]])
                    nc.gpsimd.dma_start(out=xga[:, g, :], in_=src)
            elif xmode == "pairhalf":
                for g in range(npair):
                    for hh in range(2):
                        src = bass.AP(
                            tensor=x_ext.tensor,
                            offset=x_ext.offset + g * IMPP * H * W + hh * 512,
                            ap=[[STRIDE * W, P], [1, 512]])
                        nc.gpsimd.dma_start(
                            out=xga[:, g, hh * 512:(hh + 1) * 512], in_=src)
            if stage == "dma":
                nc.vector.tensor_copy(out=s1t[:, 0:1], in_=xga[0:N, 0, 0:1])
                continue

            # Pair-sum columns on DVE: xp[p, g, w2] = x[p,2w2] + x[p,2w2+1]
            # halves A2 and every epilogue op downstream.
            xp = xpool.tile([P, npair, W // 2], bf16, tag="xp")
            for g in range(npair):
                xv2 = xga[:, g, :].rearrange("p (w two) -> p w two", two=2)
                nc.vector.tensor_tensor(
                    out=xp[:, g, :], in0=xv2[:, :, 0:1], in1=xv2[:, :, 1:2],
                    op=Alu.add)
            # all 4 matmuls first (one PSUM bank each), then epilogues:
            # ACT never waits on a matmul mid-stream
            A2s = {}
            for b in range(B_PER_CORE):
                A2 = psum.tile([N, W // 2], mybir.dt.float32, tag="A2")
                A2s[b] = A2
                nc.tensor.matmul(
                    A2[:], lhsT=ym_all[:, b, :], rhs=xp[:, b // IMPP, :],
                    start=True, stop=True)
            if stage == "mm":
                for b in range(B_PER_CORE):
                    nc.vector.tensor_copy(out=s1t[:, b:b + 1],
                                          in_=A2s[b][:, 0:1])
                continue
            for b in range(B_PER_CORE):
                # PSUM -> bf16 on ScalarE, 2x-mode masked reduce on DVE
                a2b = epool.tile([N, W // 2], bf16, tag="a2b")
                nc.scalar.activation(
                    out=a2b[:], in_=A2s[b][:],
                    func=mybir.ActivationFunctionType.Copy)
                scr = epool.tile([N, W // 2], bf16, tag="scr")
                nc.vector.scalar_tensor_tensor(
                    out=scr[:], in0=a2b[:], scalar=1.0,
                    in1=cm_all[:, b, :], op0=Alu.mult, op1=Alu.mult,
                    accum_out=s1t[:, b:b + 1])

        nc.sync.dma_start(out=s1_ext[:], in_=s1t[:])

    nc.compile()
    _CACHE[key] = nc
    return nc


def _build_phase2(repeat=1, stage="full"):
    key = ("p2", repeat, stage)
    if key in _CACHE:
        return _CACHE[key]

    from contextlib import ExitStack

    import concourse.bass as bass
    import concourse.tile as tile
    from concourse import bacc, mybir

    f32 = mybir.dt.float32
    i32 = mybir.dt.int32
    Alu = mybir.AluOpType

    nc = bacc.Bacc(None, target_bir_lowering=False, debug=False)

    x_ext = nc.dram_tensor("x", [B_PER_CORE, H, W], f32, kind="ExternalInput").ap()
    ridx_ext = nc.dram_tensor("ridx", [MAXR, 1], i32, kind="ExternalInput").ap()
    cpar_ext = nc.dram_tensor("cpar", [MAXR, 2], f32, kind="ExternalInput").ap()
    ymap_ext = nc.dram_tensor("ymap", [MAXR, MAXB], f32, kind="ExternalInput").ap()
    s2_ext = nc.dram_tensor("s2", [MAXB, 1], f32, kind="ExternalOutput").ap()

    with tile.TileContext(nc) as tc, ExitStack() as ctx:
        const = ctx.enter_context(tc.tile_pool(name="const", bufs=1))
        xpool = ctx.enter_context(tc.tile_pool(name="x", bufs=3))
        epool = ctx.enter_context(tc.tile_pool(name="epi", bufs=2))
        psum = ctx.enter_context(tc.tile_pool(name="psum", bufs=2, space="PSUM"))

        iotaw_i = const.tile([P, W], i32)
        nc.gpsimd.iota(iotaw_i[:], [[1, W]], channel_multiplier=0)
        iotaw = const.tile([P, W], f32)
        nc.vector.tensor_scalar(
            out=iotaw[:], in0=iotaw_i[:], scalar1=0.5, scalar2=None, op0=Alu.add)

        ridx = const.tile([MAXR, 1], i32)
        nc.sync.dma_start(out=ridx[:], in_=ridx_ext)
        cpar = const.tile([MAXR, 2], f32)
        nc.sync.dma_start(out=cpar[:], in_=cpar_ext)
        ymap = const.tile([MAXR, MAXB], f32)
        nc.sync.dma_start(out=ymap[:], in_=ymap_ext)

        s2t = const.tile([MAXB, 1], f32)
        x_rows = bass.AP(tensor=x_ext.tensor, offset=0,
                         ap=[[W, B_PER_CORE * H], [1, W]])

        # per-row column mask is constant per launch: build it once
        # (hidden under the first gather), not inside the repeat loop
        tcm = const.tile([MAXR, W], f32)
        nc.scalar.activation(
            out=tcm[:], in_=iotaw[0:MAXR, :],
            func=mybir.ActivationFunctionType.Abs, bias=cpar[:, 0:1],
            scale=1.0)
        cm2 = const.tile([MAXR, W], f32)
        nc.vector.tensor_scalar(
            out=cm2[:], in0=tcm[:], scalar1=cpar[:, 1:2], scalar2=None,
            op0=Alu.is_lt)

        for it in range(repeat):
            xg = xpool.tile([MAXR, W], f32, tag="xg")
            nc.gpsimd.indirect_dma_start(
                out=xg[:], out_offset=None,
                in_=x_rows,
                in_offset=bass.IndirectOffsetOnAxis(ap=ridx[:, 0:1], axis=0))
            if stage == "gather":
                nc.vector.tensor_copy(out=s2t[:, 0:1], in_=xg[0:MAXB, 0:1])
                continue
            scr = epool.tile([MAXR, W], f32, tag="scr")
            rowsum = epool.tile([MAXR, 1], f32, tag="rowsum")
            nc.vector.scalar_tensor_tensor(
                out=scr[:], in0=cm2[:], scalar=1.0, in1=xg[:],
                op0=Alu.mult, op1=Alu.mult, accum_out=rowsum[:])
            s2p = psum.tile([MAXB, 1], mybir.dt.float32, tag="s2p")
            nc.tensor.matmul(s2p[:], lhsT=ymap[:], rhs=rowsum[:],
                             start=True, stop=True)
            nc.vector.tensor_copy(out=s2t[:], in_=s2p[:])

        nc.sync.dma_start(out=s2_ext[:], in_=s2t[:])

    nc.compile()
    _CACHE[key] = nc
    return nc


def _phase2_batches(bbc, valid, s1_all):
    """Pack flagged boxes into per-core phase-2 batches (index plumbing).

    Returns (n_batches, batches) where batches[k][core] =
    (ridx [MAXR,1] i32, cpar [MAXR,2] f32, ymap [MAXR,MAXB] f32,
     slots: list of (b_global, n) per slot).
    """
    x1, y1 = bbc[..., 0], bbc[..., 1]
    x2, y2 = bbc[..., 2], bbc[..., 3]
    flag = valid & (s1_all < FLAG_TH)

    per_core = []
    for core in range(N_CORES):
        items = []  # (b_global, n, rows, mxn, rx)
        for bl in range(B_PER_CORE):
            bg = core * B_PER_CORE + bl
            for n in np.nonzero(flag[bg])[0]:
                rows = np.arange(y1[bg, n], y2[bg, n], dtype=np.int64)
                items.append((bg, n, bl * H + rows,
                              -(x1[bg, n] + x2[bg, n]) / 2.0,
                              (x2[bg, n] - x1[bg, n]) / 2.0))
        per_core.append(items)

    batches = []
    cursors = [0] * N_CORES            # (item_idx, row_offset) flattened
    row_off = [0] * N_CORES
    while True:
        any_left = any(cursors[c] < len(per_core[c]) for c in range(N_CORES))
        if not any_left and batches:
            break
        batch = []
        for core in range(N_CORES):
            ridx = np.zeros((MAXR, 1), np.int32)
            cpar = np.zeros((MAXR, 2), np.float32)
            cpar[:, 1] = -1.0          # pad rows mask to all-zero
            ymap = np.zeros((MAXR, MAXB), np.float32)
            slots = []
            r = 0
            while cursors[core] < len(per_core[core]) and len(slots) < MAXB:
                bg, n, rows, mxn, rx = per_core[core][cursors[core]]
                take = rows[row_off[core]:]
                take = take[:MAXR - r]
                if len(take) == 0:
                    break
                j = len(slots)
                slots.append((bg, n))
                ridx[r:r + len(take), 0] = take
                cpar[r:r + len(take), 0] = mxn
                cpar[r:r + len(take), 1] = rx
                ymap[r:r + len(take), j] = 1.0
                r += len(take)
                row_off[core] += len(take)
                if row_off[core] >= len(rows):
                    cursors[core] += 1
                    row_off[core] = 0
                if r >= MAXR:
                    break
            batch.append((ridx, cpar, ymap, slots))
        batches.append(batch)
        if not any_left:
            break
    return len(batches), batches


def run(output, bboxes, trace=False):
    """Run the two-phase SPMD kernel; returns (loss_scalar, results)."""
    from concourse.bass_utils import run_bass_kernel_spmd

    x_all = np.ascontiguousarray(
        output.reshape(B, H, W).astype(np.float32, copy=False))
    bb_all = np.ascontiguousarray(bboxes.astype(np.int32, copy=False))

    # --- phase 1: partial sums over sampled rows ---
    nc1 = _build_phase1()
    in_maps = []
    for i in range(N_CORES):
        sl = slice(i * B_PER_CORE, (i + 1) * B_PER_CORE)
        in_maps.append({"x": np.ascontiguousarray(x_all[sl]),
                        "bb": np.ascontiguousarray(bb_all[sl])})
    res1 = run_bass_kernel_spmd(nc1, in_maps, core_ids=list(range(N_CORES)),
                                trace=trace)
    # s1 per core: [N, B_PER_CORE] -> global [B, N]
    s1_all = np.concatenate(
        [res1.results[i]["s1"].T for i in range(N_CORES)], axis=0)

    # --- host glue: classify boxes, build phase-2 gather lists ---
    bbc = np.clip(bb_all.astype(np.int64), 0, W)
    valid = (bbc[..., 2] > bbc[..., 0]) & (bbc[..., 3] > bbc[..., 1])
    contrib = np.where(valid, 0.0, 1.0)

    n_batches, batches = _phase2_batches(bbc, valid, s1_all)

    # --- phase 2: exact sums for flagged boxes ---
    nc2 = _build_phase2()
    s2_acc = {}
    for k in range(n_batches):
        in_maps2 = []
        for core in range(N_CORES):
            ridx, cpar, ymap, _ = batches[k][core]
            in_maps2.append({
                "x": in_maps[core]["x"],
                "ridx": ridx, "cpar": cpar, "ymap": ymap})
        res2 = run_bass_kernel_spmd(nc2, in_maps2,
                                    core_ids=list(range(N_CORES)), trace=False)
        for core in range(N_CORES):
            s2 = res2.results[core]["s2"][:, 0]
            for j, (bg, n) in enumerate(batches[k][core][3]):
                s2_acc[(bg, n)] = s2_acc.get((bg, n), 0.0) + float(s2[j])

    for (bg, n), s in s2_acc.items():
        contrib[bg, n] = max(1.0 - s, 0.0)

    total = np.float32(contrib.sum(dtype=np.float64))
    return np.array(total, dtype=np.float32), (res1, n_batches)


def kernel(output, bboxes):
    loss, _ = run(output, bboxes, trace=False)
    return loss


# revision 35
# speedup vs baseline: 1.2066x; 1.1890x over previous
"""MinCountLoss Trainium2 Bass kernel — adaptive two-phase row sampling.

loss = sum_{b,n} relu(1 - box_sum(b, n)) over a non-negative density map
x: [32, 1, 1024, 1024] f32 with bboxes [32, 96, 4] (x1, y1, x2, y2).

The naive kernel streams all 128 MiB of f32 pixels from HBM and is pinned
at the chip HBM roofline (~35-49 us/iter for 16 MiB/core).  But the loss
only needs EXACT box sums for boxes whose sum is < 1; for everything else
"sum >= 1" suffices, and because the density map is non-negative, a
partial sum >= 1 PROVES the full sum is >= 1.  So:

  Phase 1 (device): read every 16th row of each image (1 MiB/core instead
    of 16 MiB/core).  For each box, compute the partial sum s1 over its
    sampled rows with the masked-row-matmul + column-mask-reduce pipeline:
      - sampled rows of 2 images pack one [128, 1024] SBUF tile (partition
        p holds image pair-half p//64, row 16*(p%64)); SWDGE casts
        f32->bf16 inside the DMA so no engine spends time casting.
      - row masks ym[p, n] and column masks cm[n, w] are precomputed once
        per launch (they depend only on bboxes).
      - A2[n, w] = sum_p ym[p, n] x[p, w] on TensorE (PSUM f32), then
        box partials via cm-masked accumulating reduce on DVE (3 images
        routed PSUM->bf16 on ScalarE first, 1 reduced straight from PSUM,
        balancing ACT/DVE at ~3 us each).
  Host glue (index plumbing only): valid boxes with s1 < 1.5 are flagged
    (margin absorbs bf16 slop; unflagged valid boxes are provably >= 1 so
    contribute exactly 0); invalid boxes contribute exactly 1.  Builds the
    phase-2 row-gather lists.
  Phase 2 (device): indirect-DMA gather of the flagged boxes' rows (f32,
    exact), mask each row to its box's x-range, reduce to exact box sums.
    Caps (128 rows / 16 boxes per core per batch) loop if ever exceeded —
    the graded workload needs 1 batch (11 flagged boxes, <=25 rows/core).

Correct for ANY non-negative density map; only the phase-2 volume is
data-dependent.  Total HBM traffic ~9 MiB vs 128 MiB for the full read.

Measured (pipelined-slope, all 8 cores sustained): phase1 ~4.3 us/iter
(DMA floor 3.3 us; DVE pair-sums columns 2:1 before the matmul so the
whole epilogue runs at half width — ACT ~2.9 us of PSUM->bf16 copies,
DVE ~3.1 us of pre-sums + masked reduces, all under the DMA) + phase2
~2.4-2.8 us (32-row indirect gather is descriptor-latency-bound), total
~7 us vs 49 us sustained (35 us paired-K) for the full-read kernel.
The pair-interior column masks keep the proof sound (s1p <= s1), at the
cost of one extra flagged box on the graded input.
"""

import numpy as np

B = 32
H = 1024
W = 1024
N = 96
N_CORES = 8
B_PER_CORE = B // N_CORES
P = 128
STRIDE = 16
SROWS = H // STRIDE          # 64 sampled rows per image
IMPP = P // SROWS            # 2 images per 128-partition tile
FLAG_TH = 1.5                # flag s1 below this (>=1 + bf16 margin)
MAXR = 32                    # gathered rows per core per phase-2 batch
MAXB = 16                    # box slots per core per phase-2 batch

_CACHE = {}


def _build_phase1(repeat=1, stage="full", xmode="pair", xbufs=3, ebufs=3,
                  pbufs=4):
    key = ("p1", repeat, stage, xmode, xbufs, ebufs, pbufs)
    if key in _CACHE:
        return _CACHE[key]

    from contextlib import ExitStack

    import concourse.bass as bass
    import concourse.tile as tile
    from concourse import bacc, mybir

    f32 = mybir.dt.float32
    bf16 = mybir.dt.bfloat16
    i32 = mybir.dt.int32
    Alu = mybir.AluOpType

    nc = bacc.Bacc(None, target_bir_lowering=False, debug=False)

    x_ext = nc.dram_tensor("x", [B_PER_CORE, H, W], f32, kind="ExternalInput").ap()
    bb_ext = nc.dram_tensor("bb", [B_PER_CORE, N, 4], i32, kind="ExternalInput").ap()
    s1_ext = nc.dram_tensor("s1", [N, B_PER_CORE], f32, kind="ExternalOutput").ap()

    with tile.TileContext(nc) as tc, ExitStack() as ctx:
        const = ctx.enter_context(tc.tile_pool(name="const", bufs=1))
        xpool = ctx.enter_context(tc.tile_pool(name="x", bufs=xbufs))
        epool = ctx.enter_context(tc.tile_pool(name="epi", bufs=ebufs))
        psum = ctx.enter_context(tc.tile_pool(name="psum", bufs=pbufs,
                                              space="PSUM"))

        # --- constants (once per launch, hidden behind the first loads) ---
        # iota2[p, w2] = 2*w2 + 0.5 (column-PAIR coordinates)
        iotaw_i = const.tile([P, W // 2], i32)
        nc.gpsimd.iota(iotaw_i[:], [[2, W // 2]], channel_multiplier=0)
        iota2 = const.tile([P, W // 2], f32)
        nc.vector.tensor_scalar(
            out=iota2[:], in0=iotaw_i[:], scalar1=0.5, scalar2=None, op0=Alu.add
        )
        # iotar[p, 1] = 16*p  (the in-pair row index of partition p)
        iotar_i = const.tile([P, 1], i32)
        nc.gpsimd.iota(iotar_i[:], [[1, 1]], channel_multiplier=STRIDE)
        iotar = const.tile([P, 1], f32)
        nc.vector.tensor_copy(out=iotar[:], in_=iotar_i[:])

        # bboxes broadcast to every partition (one contiguous DMA + cast)
        nbb = B_PER_CORE * N * 4
        bb_bc_i = const.tile([P, nbb], i32)
        bb_flat = bass.AP(tensor=bb_ext.tensor, offset=bb_ext.offset,
                          ap=[[0, P], [1, nbb]])
        nc.gpsimd.dma_start(out=bb_bc_i[:], in_=bb_flat)
        bb_bc = const.tile([P, nbb], f32)
        nc.vector.tensor_copy(out=bb_bc[:], in_=bb_bc_i[:])
        bbv = bb_bc[:].rearrange("p (b n c) -> p b n c", b=B_PER_CORE, c=4)

        # bboxes with n on partitions (for the column masks)
        bbA_i = const.tile([N, B_PER_CORE, 4], i32)
        bbA_src = bass.AP(tensor=bb_ext.tensor, offset=bb_ext.offset,
                          ap=[[4, N], [N * 4, B_PER_CORE], [1, 4]])
        nc.sync.dma_start(out=bbA_i[:], in_=bbA_src)
        bbA = const.tile([N, B_PER_CORE, 4], f32)
        nc.vector.tensor_copy(out=bbA[:], in_=bbA_i[:])

        # Row masks ym_all[p, b, n] = (y1 + 1024h <= 16p < y2 + 1024h),
        # h = b % IMPP the image's half of its pair tile.
        ym_all = const.tile([P, B_PER_CORE, N], bf16)
        for b in range(B_PER_CORE):
            h = b % IMPP
            y1v = bbv[:, b, :, 1]
            y2v = bbv[:, b, :, 3]
            if h:
                y1s = epool.tile([P, N], f32, tag="y1s")
                nc.vector.tensor_scalar(
                    out=y1s[:], in0=y1v, scalar1=float(1024 * h), scalar2=None,
                    op0=Alu.add)
                y2s = epool.tile([P, N], f32, tag="y2s")
                nc.vector.tensor_scalar(
                    out=y2s[:], in0=y2v, scalar1=float(1024 * h), scalar2=None,
                    op0=Alu.add)
                y1v, y2v = y1s[:], y2s[:]
            c2 = epool.tile([P, N], f32, tag="c2")
            nc.vector.tensor_scalar(
                out=c2[:], in0=y2v, scalar1=iotar[:], scalar2=None, op0=Alu.is_gt)
            nc.vector.scalar_tensor_tensor(
                out=ym_all[:, b, :], in0=y1v, scalar=iotar[:], in1=c2[:],
                op0=Alu.is_le, op1=Alu.mult)

        # Pair-interior column masks: cm_all[n, b, w2] = 1 iff BOTH columns
        # 2*w2, 2*w2+1 lie in [x1, x2), i.e. |2w2+0.5 - (x1+x2-1)/2| <
        # (x2-x1)/2.  Interior-only => the partial sum s1p <= s1, so
        # "s1p >= TH proves contribution 0" stays sound; the two lost edge
        # columns just flag a box or two more (sim: 12 vs 11 flagged).
        cm_all = const.tile([N, B_PER_CORE, W // 2], bf16)
        for b in range(B_PER_CORE):
            x2m1 = epool.tile([N, 1], f32, tag="x2m1")
            nc.vector.tensor_scalar(
                out=x2m1[:], in0=bbA[:, b, 2:3], scalar1=-1.0, scalar2=None,
                op0=Alu.add)
            mxn = epool.tile([N, 1], f32, tag="mxn")
            nc.vector.tensor_scalar(
                out=mxn[:], in0=bbA[:, b, 0:1], scalar1=x2m1[:],
                scalar2=-0.5, op0=Alu.add, op1=Alu.mult)
            rx = epool.tile([N, 1], f32, tag="rx")
            nc.vector.tensor_scalar(
                out=rx[:], in0=bbA[:, b, 2:3], scalar1=bbA[:, b, 0:1],
                scalar2=0.5, op0=Alu.subtract, op1=Alu.mult)
            tcm = epool.tile([N, W // 2], f32, tag="tcm")
            nc.scalar.activation(
                out=tcm[:], in_=iota2[0:N, :],
                func=mybir.ActivationFunctionType.Abs, bias=mxn[:], scale=1.0)
            nc.vector.tensor_scalar(
                out=cm_all[:, b, :], in0=tcm[:], scalar1=rx[:], scalar2=None,
                op0=Alu.is_lt)

        s1t = const.tile([N, B_PER_CORE], f32)

        npair = B_PER_CORE // IMPP
        for it in range(repeat):
            # sampled rows: partition p, pair g holds global row 16p of
            # pair block g; load path set by xmode.
            if xmode == "hw":
                # f32 on the two HWDGE rings (lower fixed latency than
                # SWDGE, no Q7); the pair-sum below casts to bf16 free.
                xga = xpool.tile([P, npair, W], f32, tag="xg")
                for g in range(npair):
                    src = bass.AP(
                        tensor=x_ext.tensor,
                        offset=x_ext.offset + g * IMPP * H * W,
                        ap=[[STRIDE * W, P], [1, W]])
                    eng = nc.sync if g == 0 else nc.scalar
                    eng.dma_start(out=xga[:, g, :], in_=src)
            else:
                xga = xpool.tile([P, npair, W], bf16, tag="xg")
            if xmode == "one":
                src = bass.AP(
                    tensor=x_ext.tensor, offset=x_ext.offset,
                    ap=[[STRIDE * W, P], [IMPP * H * W, npair], [1, W]])
                nc.gpsimd.dma_start(out=xga[:], in_=src)
            elif xmode == "pair":
                for g in range(npair):
                    src = bass.AP(
                        tensor=x_ext.tensor,
                        offset=x_ext.offset + g * IMPP * H * W,
                        ap=[[STRIDE * W, P], [1, W]])
                    nc.gpsimd.dma_start(out=xga[:, g, :], in_=src)
            elif xmode == "pairhalf":
                for g in range(npair):
                    for hh in range(2):
                        src = bass.AP(
                            tensor=x_ext.tensor,
                            offset=x_ext.offset + g * IMPP * H * W + hh * 512,
                            ap=[[STRIDE * W, P], [1, 512]])
                        nc.gpsimd.dma_start(
                            out=xga[:, g, hh * 512:(hh + 1) * 512], in_=src)
            if stage == "dma":
                nc.vector.tensor_copy(out=s1t[:, 0:1], in_=xga[0:N, 0, 0:1])
                continue

            # Column pair-sum happens FREE on TensorE: two matmuls (even
            # and odd columns, stride-2 rhs views) accumulate into the
            # same PSUM bank, so A2[n, w2] = sum_p ym*(x[2w2]+x[2w2+1])
            # with no DVE pre-sum at all.
            A2s = {}
            for b in range(B_PER_CORE):
                xv2 = xga[:, b // IMPP, :].rearrange(
                    "p (w two) -> p w two", two=2)
                A2 = psum.tile([N, W // 2], mybir.dt.float32, tag="A2")
                A2s[b] = A2
                for par in range(2):
                    nc.tensor.matmul(
                        A2[:], lhsT=ym_all[:, b, :],
                        rhs=xv2[:, :, par:par + 1].rearrange(
                            "p w one -> p (w one)"),
                        start=(par == 0), stop=(par == 1))
            if stage == "mm":
                for b in range(B_PER_CORE):
                    nc.vector.tensor_copy(out=s1t[:, b:b + 1],
                                          in_=A2s[b][:, 0:1])
                continue
            for b in range(B_PER_CORE):
                # PSUM -> bf16 on ScalarE, 2x-mode masked reduce on DVE
                a2b = epool.tile([N, W // 2], bf16, tag="a2b")
                nc.scalar.activation(
                    out=a2b[:], in_=A2s[b][:],
                    func=mybir.ActivationFunctionType.Copy)
                scr = epool.tile([N, W // 2], bf16, tag="scr")
                nc.vector.scalar_tensor_tensor(
                    out=scr[:], in0=a2b[:], scalar=1.0,
                    in1=cm_all[:, b, :], op0=Alu.mult, op1=Alu.mult,
                    accum_out=s1t[:, b:b + 1])

        nc.sync.dma_start(out=s1_ext[:], in_=s1t[:])

    nc.compile()
    _CACHE[key] = nc
    return nc


def _build_phase2(repeat=1, stage="full"):
    key = ("p2", repeat, stage)
    if key in _CACHE:
        return _CACHE[key]

    from contextlib import ExitStack

    import concourse.bass as bass
    import concourse.tile as tile
    from concourse import bacc, mybir

    f32 = mybir.dt.float32
    i32 = mybir.dt.int32
    Alu = mybir.AluOpType

    nc = bacc.Bacc(None, target_bir_lowering=False, debug=False)

    x_ext = nc.dram_tensor("x", [B_PER_CORE, H, W], f32, kind="ExternalInput").ap()
    ridx_ext = nc.dram_tensor("ridx", [MAXR, 1], i32, kind="ExternalInput").ap()
    cpar_ext = nc.dram_tensor("cpar", [MAXR, 2], f32, kind="ExternalInput").ap()
    ymap_ext = nc.dram_tensor("ymap", [MAXR, MAXB], f32, kind="ExternalInput").ap()
    s2_ext = nc.dram_tensor("s2", [MAXB, 1], f32, kind="ExternalOutput").ap()

    with tile.TileContext(nc) as tc, ExitStack() as ctx:
        const = ctx.enter_context(tc.tile_pool(name="const", bufs=1))
        xpool = ctx.enter_context(tc.tile_pool(name="x", bufs=3))
        epool = ctx.enter_context(tc.tile_pool(name="epi", bufs=2))
        psum = ctx.enter_context(tc.tile_pool(name="psum", bufs=2, space="PSUM"))

        iotaw_i = const.tile([P, W], i32)
        nc.gpsimd.iota(iotaw_i[:], [[1, W]], channel_multiplier=0)
        iotaw = const.tile([P, W], f32)
        nc.vector.tensor_scalar(
            out=iotaw[:], in0=iotaw_i[:], scalar1=0.5, scalar2=None, op0=Alu.add)

        ridx = const.tile([MAXR, 1], i32)
        nc.sync.dma_start(out=ridx[:], in_=ridx_ext)
        cpar = const.tile([MAXR, 2], f32)
        nc.sync.dma_start(out=cpar[:], in_=cpar_ext)
        ymap = const.tile([MAXR, MAXB], f32)
        nc.sync.dma_start(out=ymap[:], in_=ymap_ext)

        s2t = const.tile([MAXB, 1], f32)
        x_rows = bass.AP(tensor=x_ext.tensor, offset=0,
                         ap=[[W, B_PER_CORE * H], [1, W]])

        # per-row column mask is constant per launch: build it once
        # (hidden under the first gather), not inside the repeat loop
        tcm = const.tile([MAXR, W], f32)
        nc.scalar.activation(
            out=tcm[:], in_=iotaw[0:MAXR, :],
            func=mybir.ActivationFunctionType.Abs, bias=cpar[:, 0:1],
            scale=1.0)
        cm2 = const.tile([MAXR, W], f32)
        nc.vector.tensor_scalar(
            out=cm2[:], in0=tcm[:], scalar1=cpar[:, 1:2], scalar2=None,
            op0=Alu.is_lt)

        for it in range(repeat):
            xg = xpool.tile([MAXR, W], f32, tag="xg")
            nc.gpsimd.indirect_dma_start(
                out=xg[:], out_offset=None,
                in_=x_rows,
                in_offset=bass.IndirectOffsetOnAxis(ap=ridx[:, 0:1], axis=0))
            if stage == "gather":
                nc.vector.tensor_copy(out=s2t[:, 0:1], in_=xg[0:MAXB, 0:1])
                continue
            scr = epool.tile([MAXR, W], f32, tag="scr")
            rowsum = epool.tile([MAXR, 1], f32, tag="rowsum")
            nc.vector.scalar_tensor_tensor(
                out=scr[:], in0=cm2[:], scalar=1.0, in1=xg[:],
                op0=Alu.mult, op1=Alu.mult, accum_out=rowsum[:])
            s2p = psum.tile([MAXB, 1], mybir.dt.float32, tag="s2p")
            nc.tensor.matmul(s2p[:], lhsT=ymap[:], rhs=rowsum[:],
                             start=True, stop=True)
            nc.vector.tensor_copy(out=s2t[:], in_=s2p[:])

        nc.sync.dma_start(out=s2_ext[:], in_=s2t[:])

    nc.compile()
    _CACHE[key] = nc
    return nc


def _phase2_batches(bbc, valid, s1_all):
    """Pack flagged boxes into per-core phase-2 batches (index plumbing).

    Returns (n_batches, batches) where batches[k][core] =
    (ridx [MAXR,1] i32, cpar [MAXR,2] f32, ymap [MAXR,MAXB] f32,
     slots: list of (b_global, n) per slot).
    """
    x1, y1 = bbc[..., 0], bbc[..., 1]
    x2, y2 = bbc[..., 2], bbc[..., 3]
    flag = valid & (s1_all < FLAG_TH)

    per_core = []
    for core in range(N_CORES):
        items = []  # (b_global, n, rows, mxn, rx)
        for bl in range(B_PER_CORE):
            bg = core * B_PER_CORE + bl
            for n in np.nonzero(flag[bg])[0]:
                rows = np.arange(y1[bg, n], y2[bg, n], dtype=np.int64)
                items.append((bg, n, bl * H + rows,
                              -(x1[bg, n] + x2[bg, n]) / 2.0,
                              (x2[bg, n] - x1[bg, n]) / 2.0))
        per_core.append(items)

    batches = []
    cursors = [0] * N_CORES            # (item_idx, row_offset) flattened
    row_off = [0] * N_CORES
    while True:
        any_left = any(cursors[c] < len(per_core[c]) for c in range(N_CORES))
        if not any_left and batches:
            break
        batch = []
        for core in range(N_CORES):
            ridx = np.zeros((MAXR, 1), np.int32)
            cpar = np.zeros((MAXR, 2), np.float32)
            cpar[:, 1] = -1.0          # pad rows mask to all-zero
            ymap = np.zeros((MAXR, MAXB), np.float32)
            slots = []
            r = 0
            while cursors[core] < len(per_core[core]) and len(slots) < MAXB:
                bg, n, rows, mxn, rx = per_core[core][cursors[core]]
                take = rows[row_off[core]:]
                take = take[:MAXR - r]
                if len(take) == 0:
                    break
                j = len(slots)
                slots.append((bg, n))
                ridx[r:r + len(take), 0] = take
                cpar[r:r + len(take), 0] = mxn
                cpar[r:r + len(take), 1] = rx
                ymap[r:r + len(take), j] = 1.0
                r += len(take)
                row_off[core] += len(take)
                if row_off[core] >= len(rows):
                    cursors[core] += 1
                    row_off[core] = 0
                if r >= MAXR:
                    break
            batch.append((ridx, cpar, ymap, slots))
        batches.append(batch)
        if not any_left:
            break
    return len(batches), batches


def run(output, bboxes, trace=False):
    """Run the two-phase SPMD kernel; returns (loss_scalar, results)."""
    from concourse.bass_utils import run_bass_kernel_spmd

    x_all = np.ascontiguousarray(
        output.reshape(B, H, W).astype(np.float32, copy=False))
    bb_all = np.ascontiguousarray(bboxes.astype(np.int32, copy=False))

    # --- phase 1: partial sums over sampled rows ---
    nc1 = _build_phase1()
    in_maps = []
    for i in range(N_CORES):
        sl = slice(i * B_PER_CORE, (i + 1) * B_PER_CORE)
        in_maps.append({"x": np.ascontiguousarray(x_all[sl]),
                        "bb": np.ascontiguousarray(bb_all[sl])})
    res1 = run_bass_kernel_spmd(nc1, in_maps, core_ids=list(range(N_CORES)),
                                trace=trace)
    # s1 per core: [N, B_PER_CORE] -> global [B, N]
    s1_all = np.concatenate(
        [res1.results[i]["s1"].T for i in range(N_CORES)], axis=0)

    # --- host glue: classify boxes, build phase-2 gather lists ---
    bbc = np.clip(bb_all.astype(np.int64), 0, W)
    valid = (bbc[..., 2] > bbc[..., 0]) & (bbc[..., 3] > bbc[..., 1])
    contrib = np.where(valid, 0.0, 1.0)

    n_batches, batches = _phase2_batches(bbc, valid, s1_all)

    # --- phase 2: exact sums for flagged boxes ---
    nc2 = _build_phase2()
    s2_acc = {}
    for k in range(n_batches):
        in_maps2 = []
        for core in range(N_CORES):
            ridx, cpar, ymap, _ = batches[k][core]
            in_maps2.append({
                "x": in_maps[core]["x"],
                "ridx": ridx, "cpar": cpar, "ymap": ymap})
        res2 = run_bass_kernel_spmd(nc2, in_maps2,
                                    core_ids=list(range(N_CORES)), trace=False)
        for core in range(N_CORES):
            s2 = res2.results[core]["s2"][:, 0]
            for j, (bg, n) in enumerate(batches[k][core][3]):
                s2_acc[(bg, n)] = s2_acc.get((bg, n), 0.0) + float(s2[j])

    for (bg, n), s in s2_acc.items():
        contrib[bg, n] = max(1.0 - s, 0.0)

    total = np.float32(contrib.sum(dtype=np.float64))
    return np.array(total, dtype=np.float32), (res1, n_batches)


def kernel(output, bboxes):
    loss, _ = run(output, bboxes, trace=False)
    return loss


# revision 36
# speedup vs baseline: 1.2754x; 1.0571x over previous
"""MinCountLoss Trainium2 Bass kernel — adaptive two-phase row sampling.

loss = sum_{b,n} relu(1 - box_sum(b, n)) over a non-negative density map
x: [32, 1, 1024, 1024] f32 with bboxes [32, 96, 4] (x1, y1, x2, y2).

The naive kernel streams all 128 MiB of f32 pixels from HBM and is pinned
at the chip HBM roofline (~35-49 us/iter for 16 MiB/core).  But the loss
only needs EXACT box sums for boxes whose sum is < 1; for everything else
"sum >= 1" suffices, and because the density map is non-negative, a
partial sum >= 1 PROVES the full sum is >= 1.  So:

  Phase 1 (device): read every 16th row of each image (1 MiB/core instead
    of 16 MiB/core).  For each box, compute the partial sum s1 over its
    sampled rows with the masked-row-matmul + column-mask-reduce pipeline:
      - sampled rows of 2 images pack one [128, 1024] SBUF tile (partition
        p holds image pair-half p//64, row 16*(p%64)); SWDGE casts
        f32->bf16 inside the DMA so no engine spends time casting.
      - row masks ym[p, n] and column masks cm[n, w] are precomputed once
        per launch (they depend only on bboxes).
      - A2[n, w] = sum_p ym[p, n] x[p, w] on TensorE (PSUM f32), then
        box partials via cm-masked accumulating reduce on DVE (3 images
        routed PSUM->bf16 on ScalarE first, 1 reduced straight from PSUM,
        balancing ACT/DVE at ~3 us each).
  Host glue (index plumbing only): valid boxes with s1 < 1.5 are flagged
    (margin absorbs bf16 slop; unflagged valid boxes are provably >= 1 so
    contribute exactly 0); invalid boxes contribute exactly 1.  Builds the
    phase-2 row-gather lists.
  Phase 2 (device): indirect-DMA gather of the flagged boxes' rows (f32,
    exact), mask each row to its box's x-range, reduce to exact box sums.
    Caps (128 rows / 16 boxes per core per batch) loop if ever exceeded —
    the graded workload needs 1 batch (11 flagged boxes, <=25 rows/core).

Correct for ANY non-negative density map; only the phase-2 volume is
data-dependent.  Total HBM traffic ~9 MiB vs 128 MiB for the full read.

Measured (pipelined-slope, all 8 cores sustained): phase1 ~3.5 us/iter,
essentially AT the 3.3 us strided-DMA floor: the column pair-sum runs
free on TensorE (even/odd stride-2 rhs views accumulated into the same
PSUM bank), so the half-width epilogue leaves ACT ~2.9 us and DVE
~1.7 us fully hidden under the DMA.  Phase2 ~2.3-2.5 us (32-row
indirect gather is descriptor-latency-bound).  Total ~5.9-6.0 us vs
49 us sustained (35 us paired-K) for the full-read kernel.  The
pair-interior column masks keep the proof sound (s1p <= s1), at the
cost of one extra flagged box on the graded input.
"""

import numpy as np

B = 32
H = 1024
W = 1024
N = 96
N_CORES = 8
B_PER_CORE = B // N_CORES
P = 128
STRIDE = 16
SROWS = H // STRIDE          # 64 sampled rows per image
IMPP = P // SROWS            # 2 images per 128-partition tile
FLAG_TH = 1.5                # flag s1 below this (>=1 + bf16 margin)
MAXR = 32                    # gathered rows per core per phase-2 batch
MAXB = 16                    # box slots per core per phase-2 batch

_CACHE = {}


def _build_phase1(repeat=1, stage="full", xmode="pair", xbufs=3, ebufs=3,
                  pbufs=4):
    key = ("p1", repeat, stage, xmode, xbufs, ebufs, pbufs)
    if key in _CACHE:
        return _CACHE[key]

    from contextlib import ExitStack

    import concourse.bass as bass
    import concourse.tile as tile
    from concourse import bacc, mybir

    f32 = mybir.dt.float32
    bf16 = mybir.dt.bfloat16
    i32 = mybir.dt.int32
    Alu = mybir.AluOpType

    nc = bacc.Bacc(None, target_bir_lowering=False, debug=False)

    x_ext = nc.dram_tensor("x", [B_PER_CORE, H, W], f32, kind="ExternalInput").ap()
    bb_ext = nc.dram_tensor("bb", [B_PER_CORE, N, 4], i32, kind="ExternalInput").ap()
    s1_ext = nc.dram_tensor("s1", [N, B_PER_CORE], f32, kind="ExternalOutput").ap()

    with tile.TileContext(nc) as tc, ExitStack() as ctx:
        const = ctx.enter_context(tc.tile_pool(name="const", bufs=1))
        xpool = ctx.enter_context(tc.tile_pool(name="x", bufs=xbufs))
        epool = ctx.enter_context(tc.tile_pool(name="epi", bufs=ebufs))
        psum = ctx.enter_context(tc.tile_pool(name="psum", bufs=pbufs,
                                              space="PSUM"))

        # --- constants (once per launch, hidden behind the first loads) ---
        # iota2[p, w2] = 2*w2 + 0.5 (column-PAIR coordinates)
        iotaw_i = const.tile([P, W // 2], i32)
        nc.gpsimd.iota(iotaw_i[:], [[2, W // 2]], channel_multiplier=0)
        iota2 = const.tile([P, W // 2], f32)
        nc.vector.tensor_scalar(
            out=iota2[:], in0=iotaw_i[:], scalar1=0.5, scalar2=None, op0=Alu.add
        )
        # iotar[p, 1] = 16*p  (the in-pair row index of partition p)
        iotar_i = const.tile([P, 1], i32)
        nc.gpsimd.iota(iotar_i[:], [[1, 1]], channel_multiplier=STRIDE)
        iotar = const.tile([P, 1], f32)
        nc.vector.tensor_copy(out=iotar[:], in_=iotar_i[:])

        # bboxes broadcast to every partition (one contiguous DMA + cast)
        nbb = B_PER_CORE * N * 4
        bb_bc_i = const.tile([P, nbb], i32)
        bb_flat = bass.AP(tensor=bb_ext.tensor, offset=bb_ext.offset,
                          ap=[[0, P], [1, nbb]])
        nc.gpsimd.dma_start(out=bb_bc_i[:], in_=bb_flat)
        bb_bc = const.tile([P, nbb], f32)
        nc.vector.tensor_copy(out=bb_bc[:], in_=bb_bc_i[:])
        bbv = bb_bc[:].rearrange("p (b n c) -> p b n c", b=B_PER_CORE, c=4)

        # bboxes with n on partitions (for the column masks)
        bbA_i = const.tile([N, B_PER_CORE, 4], i32)
        bbA_src = bass.AP(tensor=bb_ext.tensor, offset=bb_ext.offset,
                          ap=[[4, N], [N * 4, B_PER_CORE], [1, 4]])
        nc.sync.dma_start(out=bbA_i[:], in_=bbA_src)
        bbA = const.tile([N, B_PER_CORE, 4], f32)
        nc.vector.tensor_copy(out=bbA[:], in_=bbA_i[:])

        # Row masks ym_all[p, b, n] = (y1 + 1024h <= 16p < y2 + 1024h),
        # h = b % IMPP the image's half of its pair tile.
        ym_all = const.tile([P, B_PER_CORE, N], bf16)
        for b in range(B_PER_CORE):
            h = b % IMPP
            y1v = bbv[:, b, :, 1]
            y2v = bbv[:, b, :, 3]
            if h:
                y1s = epool.tile([P, N], f32, tag="y1s")
                nc.vector.tensor_scalar(
                    out=y1s[:], in0=y1v, scalar1=float(1024 * h), scalar2=None,
                    op0=Alu.add)
                y2s = epool.tile([P, N], f32, tag="y2s")
                nc.vector.tensor_scalar(
                    out=y2s[:], in0=y2v, scalar1=float(1024 * h), scalar2=None,
                    op0=Alu.add)
                y1v, y2v = y1s[:], y2s[:]
            c2 = epool.tile([P, N], f32, tag="c2")
            nc.vector.tensor_scalar(
                out=c2[:], in0=y2v, scalar1=iotar[:], scalar2=None, op0=Alu.is_gt)
            nc.vector.scalar_tensor_tensor(
                out=ym_all[:, b, :], in0=y1v, scalar=iotar[:], in1=c2[:],
                op0=Alu.is_le, op1=Alu.mult)

        # Pair-interior column masks: cm_all[n, b, w2] = 1 iff BOTH columns
        # 2*w2, 2*w2+1 lie in [x1, x2), i.e. |2w2+0.5 - (x1+x2-1)/2| <
        # (x2-x1)/2.  Interior-only => the partial sum s1p <= s1, so
        # "s1p >= TH proves contribution 0" stays sound; the two lost edge
        # columns just flag a box or two more (sim: 12 vs 11 flagged).
        cm_all = const.tile([N, B_PER_CORE, W // 2], bf16)
        for b in range(B_PER_CORE):
            x2m1 = epool.tile([N, 1], f32, tag="x2m1")
            nc.vector.tensor_scalar(
                out=x2m1[:], in0=bbA[:, b, 2:3], scalar1=-1.0, scalar2=None,
                op0=Alu.add)
            mxn = epool.tile([N, 1], f32, tag="mxn")
            nc.vector.tensor_scalar(
                out=mxn[:], in0=bbA[:, b, 0:1], scalar1=x2m1[:],
                scalar2=-0.5, op0=Alu.add, op1=Alu.mult)
            rx = epool.tile([N, 1], f32, tag="rx")
            nc.vector.tensor_scalar(
                out=rx[:], in0=bbA[:, b, 2:3], scalar1=bbA[:, b, 0:1],
                scalar2=0.5, op0=Alu.subtract, op1=Alu.mult)
            tcm = epool.tile([N, W // 2], f32, tag="tcm")
            nc.scalar.activation(
                out=tcm[:], in_=iota2[0:N, :],
                func=mybir.ActivationFunctionType.Abs, bias=mxn[:], scale=1.0)
            nc.vector.tensor_scalar(
                out=cm_all[:, b, :], in0=tcm[:], scalar1=rx[:], scalar2=None,
                op0=Alu.is_lt)

        s1t = const.tile([N, B_PER_CORE], f32)

        npair = B_PER_CORE // IMPP
        for it in range(repeat):
            # sampled rows: partition p, pair g holds global row 16p of
            # pair block g; load path set by xmode.
            if xmode == "hw":
                # f32 on the two HWDGE rings (lower fixed latency than
                # SWDGE, no Q7); the pair-sum below casts to bf16 free.
                xga = xpool.tile([P, npair, W], f32, tag="xg")
                for g in range(npair):
                    src = bass.AP(
                        tensor=x_ext.tensor,
                        offset=x_ext.offset + g * IMPP * H * W,
                        ap=[[STRIDE * W, P], [1, W]])
                    eng = nc.sync if g == 0 else nc.scalar
                    eng.dma_start(out=xga[:, g, :], in_=src)
            else:
                xga = xpool.tile([P, npair, W], bf16, tag="xg")
            if xmode == "one":
                src = bass.AP(
                    tensor=x_ext.tensor, offset=x_ext.offset,
                    ap=[[STRIDE * W, P], [IMPP * H * W, npair], [1, W]])
                nc.gpsimd.dma_start(out=xga[:], in_=src)
            elif xmode == "pair":
                for g in range(npair):
                    src = bass.AP(
                        tensor=x_ext.tensor,
                        offset=x_ext.offset + g * IMPP * H * W,
                        ap=[[STRIDE * W, P], [1, W]])
                    nc.gpsimd.dma_start(out=xga[:, g, :], in_=src)
            elif xmode == "pairhalf":
                for g in range(npair):
                    for hh in range(2):
                        src = bass.AP(
                            tensor=x_ext.tensor,
                            offset=x_ext.offset + g * IMPP * H * W + hh * 512,
                            ap=[[STRIDE * W, P], [1, 512]])
                        nc.gpsimd.dma_start(
                            out=xga[:, g, hh * 512:(hh + 1) * 512], in_=src)
            if stage == "dma":
                nc.vector.tensor_copy(out=s1t[:, 0:1], in_=xga[0:N, 0, 0:1])
                continue

            # Column pair-sum happens FREE on TensorE: two matmuls (even
            # and odd columns, stride-2 rhs views) accumulate into the
            # same PSUM bank, so A2[n, w2] = sum_p ym*(x[2w2]+x[2w2+1])
            # with no DVE pre-sum at all.
            A2s = {}
            for b in range(B_PER_CORE):
                xv2 = xga[:, b // IMPP, :].rearrange(
                    "p (w two) -> p w two", two=2)
                A2 = psum.tile([N, W // 2], mybir.dt.float32, tag="A2")
                A2s[b] = A2
                for par in range(2):
                    nc.tensor.matmul(
                        A2[:], lhsT=ym_all[:, b, :],
                        rhs=xv2[:, :, par:par + 1].rearrange(
                            "p w one -> p (w one)"),
                        start=(par == 0), stop=(par == 1))
            if stage == "mm":
                for b in range(B_PER_CORE):
                    nc.vector.tensor_copy(out=s1t[:, b:b + 1],
                                          in_=A2s[b][:, 0:1])
                continue
            for b in range(B_PER_CORE):
                # PSUM -> bf16 on ScalarE, 2x-mode masked reduce on DVE
                a2b = epool.tile([N, W // 2], bf16, tag="a2b")
                nc.scalar.activation(
                    out=a2b[:], in_=A2s[b][:],
                    func=mybir.ActivationFunctionType.Copy)
                scr = epool.tile([N, W // 2], bf16, tag="scr")
                nc.vector.scalar_tensor_tensor(
                    out=scr[:], in0=a2b[:], scalar=1.0,
                    in1=cm_all[:, b, :], op0=Alu.mult, op1=Alu.mult,
                    accum_out=s1t[:, b:b + 1])

        nc.sync.dma_start(out=s1_ext[:], in_=s1t[:])

    nc.compile()
    _CACHE[key] = nc
    return nc


def _build_phase2(repeat=1, stage="full"):
    key = ("p2", repeat, stage)
    if key in _CACHE:
        return _CACHE[key]

    from contextlib import ExitStack

    import concourse.bass as bass
    import concourse.tile as tile
    from concourse import bacc, mybir

    f32 = mybir.dt.float32
    i32 = mybir.dt.int32
    Alu = mybir.AluOpType

    nc = bacc.Bacc(None, target_bir_lowering=False, debug=False)

    x_ext = nc.dram_tensor("x", [B_PER_CORE, H, W], f32, kind="ExternalInput").ap()
    ridx_ext = nc.dram_tensor("ridx", [MAXR, 1], i32, kind="ExternalInput").ap()
    cpar_ext = nc.dram_tensor("cpar", [MAXR, 2], f32, kind="ExternalInput").ap()
    ymap_ext = nc.dram_tensor("ymap", [MAXR, MAXB], f32, kind="ExternalInput").ap()
    s2_ext = nc.dram_tensor("s2", [MAXB, 1], f32, kind="ExternalOutput").ap()

    with tile.TileContext(nc) as tc, ExitStack() as ctx:
        const = ctx.enter_context(tc.tile_pool(name="const", bufs=1))
        xpool = ctx.enter_context(tc.tile_pool(name="x", bufs=3))
        epool = ctx.enter_context(tc.tile_pool(name="epi", bufs=2))
        psum = ctx.enter_context(tc.tile_pool(name="psum", bufs=2, space="PSUM"))

        iotaw_i = const.tile([P, W], i32)
        nc.gpsimd.iota(iotaw_i[:], [[1, W]], channel_multiplier=0)
        iotaw = const.tile([P, W], f32)
        nc.vector.tensor_scalar(
            out=iotaw[:], in0=iotaw_i[:], scalar1=0.5, scalar2=None, op0=Alu.add)

        ridx = const.tile([MAXR, 1], i32)
        nc.sync.dma_start(out=ridx[:], in_=ridx_ext)
        cpar = const.tile([MAXR, 2], f32)
        nc.sync.dma_start(out=cpar[:], in_=cpar_ext)
        ymap = const.tile([MAXR, MAXB], f32)
        nc.sync.dma_start(out=ymap[:], in_=ymap_ext)

        s2t = const.tile([MAXB, 1], f32)
        x_rows = bass.AP(tensor=x_ext.tensor, offset=0,
                         ap=[[W, B_PER_CORE * H], [1, W]])

        # per-row column mask is constant per launch: build it once
        # (hidden under the first gather), not inside the repeat loop
        tcm = const.tile([MAXR, W], f32)
        nc.scalar.activation(
            out=tcm[:], in_=iotaw[0:MAXR, :],
            func=mybir.ActivationFunctionType.Abs, bias=cpar[:, 0:1],
            scale=1.0)
        cm2 = const.tile([MAXR, W], f32)
        nc.vector.tensor_scalar(
            out=cm2[:], in0=tcm[:], scalar1=cpar[:, 1:2], scalar2=None,
            op0=Alu.is_lt)

        for it in range(repeat):
            xg = xpool.tile([MAXR, W], f32, tag="xg")
            nc.gpsimd.indirect_dma_start(
                out=xg[:], out_offset=None,
                in_=x_rows,
                in_offset=bass.IndirectOffsetOnAxis(ap=ridx[:, 0:1], axis=0))
            if stage == "gather":
                nc.vector.tensor_copy(out=s2t[:, 0:1], in_=xg[0:MAXB, 0:1])
                continue
            scr = epool.tile([MAXR, W], f32, tag="scr")
            rowsum = epool.tile([MAXR, 1], f32, tag="rowsum")
            nc.vector.scalar_tensor_tensor(
                out=scr[:], in0=cm2[:], scalar=1.0, in1=xg[:],
                op0=Alu.mult, op1=Alu.mult, accum_out=rowsum[:])
            s2p = psum.tile([MAXB, 1], mybir.dt.float32, tag="s2p")
            nc.tensor.matmul(s2p[:], lhsT=ymap[:], rhs=rowsum[:],
                             start=True, stop=True)
            nc.vector.tensor_copy(out=s2t[:], in_=s2p[:])

        nc.sync.dma_start(out=s2_ext[:], in_=s2t[:])

    nc.compile()
    _CACHE[key] = nc
    return nc


def _phase2_batches(bbc, valid, s1_all):
    """Pack flagged boxes into per-core phase-2 batches (index plumbing).

    Returns (n_batches, batches) where batches[k][core] =
    (ridx [MAXR,1] i32, cpar [MAXR,2] f32, ymap [MAXR,MAXB] f32,
     slots: list of (b_global, n) per slot).
    """
    x1, y1 = bbc[..., 0], bbc[..., 1]
    x2, y2 = bbc[..., 2], bbc[..., 3]
    flag = valid & (s1_all < FLAG_TH)

    per_core = []
    for core in range(N_CORES):
        items = []  # (b_global, n, rows, mxn, rx)
        for bl in range(B_PER_CORE):
            bg = core * B_PER_CORE + bl
            for n in np.nonzero(flag[bg])[0]:
                rows = np.arange(y1[bg, n], y2[bg, n], dtype=np.int64)
                items.append((bg, n, bl * H + rows,
                              -(x1[bg, n] + x2[bg, n]) / 2.0,
                              (x2[bg, n] - x1[bg, n]) / 2.0))
        per_core.append(items)

    batches = []
    cursors = [0] * N_CORES            # (item_idx, row_offset) flattened
    row_off = [0] * N_CORES
    while True:
        any_left = any(cursors[c] < len(per_core[c]) for c in range(N_CORES))
        if not any_left and batches:
            break
        batch = []
        for core in range(N_CORES):
            ridx = np.zeros((MAXR, 1), np.int32)
            cpar = np.zeros((MAXR, 2), np.float32)
            cpar[:, 1] = -1.0          # pad rows mask to all-zero
            ymap = np.zeros((MAXR, MAXB), np.float32)
            slots = []
            r = 0
            while cursors[core] < len(per_core[core]) and len(slots) < MAXB:
                bg, n, rows, mxn, rx = per_core[core][cursors[core]]
                take = rows[row_off[core]:]
                take = take[:MAXR - r]
                if len(take) == 0:
                    break
                j = len(slots)
                slots.append((bg, n))
                ridx[r:r + len(take), 0] = take
                cpar[r:r + len(take), 0] = mxn
                cpar[r:r + len(take), 1] = rx
                ymap[r:r + len(take), j] = 1.0
                r += len(take)
                row_off[core] += len(take)
                if row_off[core] >= len(rows):
                    cursors[core] += 1
                    row_off[core] = 0
                if r >= MAXR:
                    break
            batch.append((ridx, cpar, ymap, slots))
        batches.append(batch)
        if not any_left:
            break
    return len(batches), batches


def run(output, bboxes, trace=False):
    """Run the two-phase SPMD kernel; returns (loss_scalar, results)."""
    from concourse.bass_utils import run_bass_kernel_spmd

    x_all = np.ascontiguousarray(
        output.reshape(B, H, W).astype(np.float32, copy=False))
    bb_all = np.ascontiguousarray(bboxes.astype(np.int32, copy=False))

    # --- phase 1: partial sums over sampled rows ---
    nc1 = _build_phase1()
    in_maps = []
    for i in range(N_CORES):
        sl = slice(i * B_PER_CORE, (i + 1) * B_PER_CORE)
        in_maps.append({"x": np.ascontiguousarray(x_all[sl]),
                        "bb": np.ascontiguousarray(bb_all[sl])})
    res1 = run_bass_kernel_spmd(nc1, in_maps, core_ids=list(range(N_CORES)),
                                trace=trace)
    # s1 per core: [N, B_PER_CORE] -> global [B, N]
    s1_all = np.concatenate(
        [res1.results[i]["s1"].T for i in range(N_CORES)], axis=0)

    # --- host glue: classify boxes, build phase-2 gather lists ---
    bbc = np.clip(bb_all.astype(np.int64), 0, W)
    valid = (bbc[..., 2] > bbc[..., 0]) & (bbc[..., 3] > bbc[..., 1])
    contrib = np.where(valid, 0.0, 1.0)

    n_batches, batches = _phase2_batches(bbc, valid, s1_all)

    # --- phase 2: exact sums for flagged boxes ---
    nc2 = _build_phase2()
    s2_acc = {}
    for k in range(n_batches):
        in_maps2 = []
        for core in range(N_CORES):
            ridx, cpar, ymap, _ = batches[k][core]
            in_maps2.append({
                "x": in_maps[core]["x"],
                "ridx": ridx, "cpar": cpar, "ymap": ymap})
        res2 = run_bass_kernel_spmd(nc2, in_maps2,
                                    core_ids=list(range(N_CORES)), trace=False)
        for core in range(N_CORES):
            s2 = res2.results[core]["s2"][:, 0]
            for j, (bg, n) in enumerate(batches[k][core][3]):
                s2_acc[(bg, n)] = s2_acc.get((bg, n), 0.0) + float(s2[j])

    for (bg, n), s in s2_acc.items():
        contrib[bg, n] = max(1.0 - s, 0.0)

    total = np.float32(contrib.sum(dtype=np.float64))
    return np.array(total, dtype=np.float32), (res1, n_batches)


def kernel(output, bboxes):
    loss, _ = run(output, bboxes, trace=False)
    return loss
